# revision 2
# baseline (speedup 1.0000x reference)
"""Trainium2 Bass kernel for the masked note-accuracy loss.

Reference math (per sequence n):
    pred      = (sigmoid(x) > 0.5) = (x > 0)
    tru_pos_n = sum_{t,d} pred * target
    A[n,t]    = false_pos + false_neg = sum_d (pred + target - 2*pred*target)
              = sum_d |pred - target|          (target in [0,1])
    ratio     = tru_pos_n / (tru_pos_n + A[n,t])
    acc_n     = sum_{t < T_n} ratio / T_n,   T_n = sum_t mask[n,t]
    out       = sum_n acc_n

Sharding: data-parallel over N=128 sequences -> 16 per core on 8 cores.
Each core computes its partial sum of acc_n; the host sums the 8 partials.

Per-core layout: each sequence [T=2048, D=88] is loaded as a [128, 16, 88]
tile (partition p holds rows t = p*16+k, contiguous 5632B per partition).
Derived per-(n,t) stats live as [128, n, k] with t = p*16 + k.

tru_pos is recovered from per-partition running sums without a third big
DVE pass:  S = (P + Q - A)/2, where (P-Q) comes free as the accum_out of
the V = pred - target pass (DVE), Q = sum(target) comes from a Copy
activation with accum_out on the scalar engine (runs in parallel), and
A_tot is the total of the per-t A values.
"""

import numpy as np

import concourse.bacc as bacc
import concourse.tile as tile
from concourse import mybir
from concourse.alu_op_type import AluOpType
from concourse.bass_utils import run_bass_kernel_spmd

N, T, D = 128, 2048, 88
N_CORES = 8
NS = N // N_CORES          # sequences per core
P = 128                    # partitions
K = T // P                 # rows (t values) per partition

_cached_nc = None


def _build():
    f32 = mybir.dt.float32
    nc = bacc.Bacc("TRN2", target_bir_lowering=False, debug=False,
                   num_devices=N_CORES)
    xd = nc.dram_tensor("output", [NS, T, D], f32, kind="ExternalInput")
    yd = nc.dram_tensor("target", [NS, T, D], f32, kind="ExternalInput")
    md = nc.dram_tensor("mask", [NS, T], mybir.dt.int32, kind="ExternalInput")
    od = nc.dram_tensor("partial", [1, 1], f32, kind="ExternalOutput")

    with tile.TileContext(nc) as tc:
        with (
            tc.tile_pool(name="data", bufs=3) as data_pool,
            tc.tile_pool(name="work", bufs=2) as work_pool,
            tc.tile_pool(name="singles", bufs=1) as singles,
            tc.tile_pool(name="psum", bufs=1, space="PSUM") as psum,
        ):
            # persistent stats, written slice-wise in the loop
            statA = singles.tile([P, NS, K], f32)    # A[n,t]
            pmq = singles.tile([P, NS], f32)         # per-partition P-Q
            qp = singles.tile([P, NS], f32)          # per-partition Q
            maskf = singles.tile([P, NS, K], f32)
            ones_col = singles.tile([P, 1], f32)
            ones_row = singles.tile([1, P], f32)
            nc.vector.memset(ones_col[:], 1.0)
            nc.vector.memset(ones_row[:], 1.0)

            maski = singles.tile([P, NS, K], mybir.dt.int32)
            nc.sync.dma_start(maski[:], md.ap().rearrange("n (p k) -> p n k", p=P))
            nc.vector.tensor_copy(maskf[:], maski[:])

            for n in range(NS):
                xt = data_pool.tile([P, K, D], f32, tag="xt")
                yt = data_pool.tile([P, K, D], f32, tag="yt")
                nc.sync.dma_start(xt[:], xd.ap()[n].rearrange("(p k) d -> p k d", p=P))
                nc.sync.dma_start(yt[:], yd.ap()[n].rearrange("(p k) d -> p k d", p=P))

                # V = (x > 0) - target ; accum: P-Q per partition
                v = work_pool.tile([P, K, D], f32, tag="v")
                nc.vector.scalar_tensor_tensor(
                    out=v[:], in0=xt[:], scalar=0.0, in1=yt[:],
                    op0=AluOpType.is_gt, op1=AluOpType.subtract,
                    accum_out=pmq[:, n : n + 1],
                )
                # A[n, t] = sum_d |V|
                nc.vector.tensor_reduce(
                    out=statA[:, n, :], in_=v[:],
                    axis=mybir.AxisListType.X, op=AluOpType.add,
                    apply_absolute_value=True,
                )
                # Q per partition via scalar engine (parallel with DVE)
                scratch = work_pool.tile([P, K, D], f32, tag="scratch")
                nc.scalar.activation(
                    out=scratch[:], in_=yt[:],
                    func=mybir.ActivationFunctionType.Copy,
                    accum_out=qp[:, n : n + 1],
                )

            # ---- tiny epilogue ----
            # column sums over partitions via PE
            ps_a = psum.tile([1, NS * K], f32)
            ps_m = psum.tile([1, NS * K], f32)
            ps_pq = psum.tile([1, NS], f32)
            ps_q = psum.tile([1, NS], f32)
            nc.tensor.matmul(ps_a[:], ones_col[:],
                             statA[:].rearrange("p a b -> p (a b)"))
            nc.tensor.matmul(ps_m[:], ones_col[:],
                             maskf[:].rearrange("p a b -> p (a b)"))
            nc.tensor.matmul(ps_pq[:], ones_col[:], pmq[:])
            nc.tensor.matmul(ps_q[:], ones_col[:], qp[:])

            row_at = singles.tile([1, NS], f32)   # A_tot per seq
            nc.vector.tensor_reduce(
                out=row_at[:], in_=ps_a[:].rearrange("o (a b) -> o a b", a=NS),
                axis=mybir.AxisListType.X, op=AluOpType.add)
            row_ti = singles.tile([1, NS], f32)   # T_i per seq
            nc.vector.tensor_reduce(
                out=row_ti[:], in_=ps_m[:].rearrange("o (a b) -> o a b", a=NS),
                axis=mybir.AxisListType.X, op=AluOpType.add)

            # tru_pos = 0.5 * ((P-Q) + 2Q - A_tot)
            row_pq = singles.tile([1, NS], f32)
            nc.vector.tensor_copy(row_pq[:], ps_pq[:])
            row_tmp = singles.tile([1, NS], f32)
            nc.vector.scalar_tensor_tensor(
                out=row_tmp[:], in0=ps_q[:], scalar=2.0, in1=row_pq[:],
                op0=AluOpType.mult, op1=AluOpType.add)
            row_s2 = singles.tile([1, NS], f32)
            nc.vector.tensor_sub(row_s2[:], row_tmp[:], row_at[:])
            row_s = singles.tile([1, NS], f32)
            nc.vector.tensor_scalar_mul(row_s[:], row_s2[:], 0.5)

            inv_ti = singles.tile([1, NS], f32)
            nc.vector.reciprocal(inv_ti[:], row_ti[:])

            # broadcast tru_pos to all partitions
            ps_bs = psum.tile([P, NS], f32)
            nc.tensor.matmul(ps_bs[:], ones_row[:], row_s[:])
            sb_bs = singles.tile([P, NS], f32)
            nc.vector.tensor_copy(sb_bs[:], ps_bs[:])

            # ratio = tru_pos / (tru_pos + A) * mask
            den = singles.tile([P, NS, K], f32)
            for n in range(NS):
                nc.vector.tensor_scalar_add(
                    den[:, n, :], statA[:, n, :], sb_bs[:, n : n + 1])
            rec = singles.tile([P, NS, K], f32)
            nc.vector.reciprocal(rec[:], den[:])
            rat = singles.tile([P, NS, K], f32)
            for n in range(NS):
                nc.vector.scalar_tensor_tensor(
                    out=rat[:, n, :], in0=rec[:, n, :],
                    scalar=sb_bs[:, n : n + 1], in1=maskf[:, n, :],
                    op0=AluOpType.mult, op1=AluOpType.mult)

            # per-seq sums of ratio, / T_i, then total
            ps_r = psum.tile([1, NS * K], f32)
            nc.tensor.matmul(ps_r[:], ones_col[:],
                             rat[:].rearrange("p a b -> p (a b)"))
            row_sr = singles.tile([1, NS], f32)
            nc.vector.tensor_reduce(
                out=row_sr[:], in_=ps_r[:].rearrange("o (a b) -> o a b", a=NS),
                axis=mybir.AxisListType.X, op=AluOpType.add)
            row_acc = singles.tile([1, NS], f32)
            nc.vector.tensor_mul(row_acc[:], row_sr[:], inv_ti[:])
            row_tot = singles.tile([1, 1], f32)
            nc.vector.tensor_reduce(
                out=row_tot[:], in_=row_acc[:],
                axis=mybir.AxisListType.X, op=AluOpType.add)
            nc.sync.dma_start(od.ap(), row_tot[:])

    nc.compile()
    return nc


def kernel(output, target, mask):
    global _cached_nc
    if _cached_nc is None:
        _cached_nc = _build()
    nc = _cached_nc
    output = np.asarray(output, dtype=np.float32)
    target = np.asarray(target, dtype=np.float32)
    mask = np.asarray(mask, dtype=np.int32)
    in_maps = []
    for c in range(N_CORES):
        sl = slice(c * NS, (c + 1) * NS)
        in_maps.append({
            "output": np.ascontiguousarray(output[sl]),
            "target": np.ascontiguousarray(target[sl]),
            "mask": np.ascontiguousarray(mask[sl]),
        })
    res = run_bass_kernel_spmd(nc, in_maps, list(range(N_CORES)))
    total = np.float32(0.0)
    for c in range(N_CORES):
        total = np.float32(total + np.float32(res.results[c]["partial"].reshape(())))
    return np.float32(total)


# revision 24
# speedup vs baseline: 1.0675x; 1.0675x over previous
"""Trainium2 Bass kernel for the masked note-accuracy loss (best-measured config)."""

import numpy as np

import concourse.bacc as bacc
import concourse.tile as tile
from concourse import mybir
from concourse.alu_op_type import AluOpType
from concourse.bass_utils import run_bass_kernel_spmd

N, T, D = 128, 2048, 88
N_CORES = 8
NS = N // N_CORES
P = 128
K = T // P

_cached_nc = None

USE_BF16_V = True
USE_NEGATE = True


def _build():
    f32 = mybir.dt.float32
    vdt = mybir.dt.bfloat16 if USE_BF16_V else f32
    nc = bacc.Bacc("TRN2", target_bir_lowering=False, debug=False,
                   num_devices=N_CORES)
    xd = nc.dram_tensor("output", [NS, T, D], f32, kind="ExternalInput")
    yd = nc.dram_tensor("target", [NS, T, D], f32, kind="ExternalInput")
    md = nc.dram_tensor("mask", [NS, T], mybir.dt.int32, kind="ExternalInput")
    od = nc.dram_tensor("partial", [1, 1], f32, kind="ExternalOutput")

    AX = mybir.AxisListType.X

    with tile.TileContext(nc) as tc:
        with (
            tc.tile_pool(name="data", bufs=3) as data_pool,
            tc.tile_pool(name="work", bufs=2) as work_pool,
            tc.tile_pool(name="mini", bufs=2) as mini_pool,
            tc.tile_pool(name="singles", bufs=1) as singles,
            tc.tile_pool(name="psl", bufs=2, space="PSUM") as psum_loop,
            tc.tile_pool(name="psk", bufs=1, space="PSUM") as psum_keep,
        ):
            stA = singles.tile([P, NS, 16], f32)
            stPQ = singles.tile([P, NS], f32)
            stQ2 = singles.tile([P, NS], f32)
            maskf = singles.tile([P, NS, K], f32)
            maski = singles.tile([P, NS, K], mybir.dt.int32)
            ones_col = singles.tile([P, 1], f32)
            ones_row = singles.tile([1, P], f32)
            inv_ti = singles.tile([1, NS], f32)
            row_ti = singles.tile([1, NS], f32)
            nc.vector.memset(ones_col[:], 1.0)
            nc.vector.memset(ones_row[:], 1.0)

            ps_m = psum_keep.tile([1, NS * K], f32)

            acc0 = mini_pool.tile([1, 1], f32, tag="acc")
            acc_prev = [acc0]
            nc.vector.memset(acc_prev[0][:], 0.0)

            def load(n):
                xt = data_pool.tile([P, K, D], f32, tag="xt")
                yt = data_pool.tile([P, K, D], f32, tag="yt")
                nc.sync.dma_start(xt[:], xd.ap()[n].rearrange("(p k) d -> p k d", p=P))
                nc.scalar.dma_start(yt[:], yd.ap()[n].rearrange("(p k) d -> p k d", p=P))
                return xt, yt

            def compute(n, xt, yt):
                v = work_pool.tile([P, K, D], vdt, tag="v")
                nc.vector.scalar_tensor_tensor(
                    out=v[:], in0=xt[:], scalar=0.0, in1=yt[:],
                    op0=AluOpType.is_gt, op1=AluOpType.subtract,
                    accum_out=stPQ[:, n : n + 1],
                )
                nc.vector.tensor_reduce(
                    out=stA[:, n, :], in_=v[:], axis=AX, op=AluOpType.add,
                    apply_absolute_value=True, negate=USE_NEGATE,
                )
                scratch = work_pool.tile([P, K, D], vdt, tag="scratch")
                nc.scalar.activation(
                    out=scratch[:], in_=yt[:],
                    func=mybir.ActivationFunctionType.Copy, scale=2.0,
                    accum_out=stQ2[:, n : n + 1],
                )
                # ---- mini epilogue ----
                ps_st = psum_loop.tile([1, 18], f32, tag="ps_st")
                nc.tensor.matmul(ps_st[0:1, 0:16], ones_col[:], stA[:, n, :])
                nc.tensor.matmul(ps_st[0:1, 16:17], ones_col[:],
                                 stPQ[:, n : n + 1])
                nc.tensor.matmul(ps_st[0:1, 17:18], ones_col[:],
                                 stQ2[:, n : n + 1])
                row_s2 = mini_pool.tile([1, 1], f32, tag="row_s2")
                nc.vector.tensor_reduce(
                    out=row_s2[:], in_=ps_st[:], axis=AX, op=AluOpType.add)
                ps_b = psum_loop.tile([P, 1], f32, tag="ps_b")
                nc.tensor.matmul(ps_b[:], ones_row[:], row_s2[:])
                sb_b = mini_pool.tile([P, 1], f32, tag="sb_b")
                nc.vector.tensor_copy(sb_b[:], ps_b[:])
                den = mini_pool.tile([P, K], f32, tag="den")
                nc.vector.tensor_scalar(
                    out=den[:], in0=stA[:, n, :],
                    scalar1=-2.0, scalar2=sb_b[:], op0=AluOpType.mult,
                    op1=AluOpType.add)
                rec = mini_pool.tile([P, K], f32, tag="rec")
                nc.vector.reciprocal(rec[:], den[:])
                rat = mini_pool.tile([P, K], f32, tag="rat")
                nc.vector.scalar_tensor_tensor(
                    out=rat[:], in0=rec[:], scalar=sb_b[:],
                    in1=maskf[:, n, :],
                    op0=AluOpType.mult, op1=AluOpType.mult)
                ps_rat = psum_loop.tile([1, K], f32, tag="ps_rat")
                nc.tensor.matmul(ps_rat[:], ones_col[:], rat[:])
                row_c = mini_pool.tile([1, 1], f32, tag="row_c")
                nc.vector.tensor_reduce(
                    out=row_c[:], in_=ps_rat[:], axis=AX, op=AluOpType.add)
                acc_new = mini_pool.tile([1, 1], f32, tag="acc")
                nc.vector.scalar_tensor_tensor(
                    out=acc_new[:], in0=row_c[:],
                    scalar=inv_ti[0:1, n : n + 1], in1=acc_prev[0][:],
                    op0=AluOpType.mult, op1=AluOpType.add)
                acc_prev[0] = acc_new

            xt0, yt0 = load(0)
            nc.gpsimd.dma_start(maski[:], md.ap().rearrange("n (p k) -> p n k", p=P))
            nc.vector.tensor_copy(maskf[:], maski[:])
            nc.tensor.matmul(ps_m[:], ones_col[:],
                             maskf[:].rearrange("p a b -> p (a b)"))
            nc.vector.tensor_reduce(
                out=row_ti[:], in_=ps_m[:].rearrange("o (a b) -> o a b", a=NS),
                axis=AX, op=AluOpType.add)
            nc.vector.reciprocal(inv_ti[:], row_ti[:])

            compute(0, xt0, yt0)
            for n in range(1, NS):
                xt, yt = load(n)
                compute(n, xt, yt)

            nc.sync.dma_start(od.ap(), acc_prev[0][:])

    nc.compile()
    return nc


def kernel(output, target, mask):
    global _cached_nc
    if _cached_nc is None:
        _cached_nc = _build()
    nc = _cached_nc
    output = np.asarray(output, dtype=np.float32)
    target = np.asarray(target, dtype=np.float32)
    mask = np.asarray(mask, dtype=np.int32)
    in_maps = []
    for c in range(N_CORES):
        sl = slice(c * NS, (c + 1) * NS)
        in_maps.append({
            "output": np.ascontiguousarray(output[sl]),
            "target": np.ascontiguousarray(target[sl]),
            "mask": np.ascontiguousarray(mask[sl]),
        })
    res = run_bass_kernel_spmd(nc, in_maps, list(range(N_CORES)))
    total = np.float32(0.0)
    for c in range(N_CORES):
        total = np.float32(total + np.float32(res.results[c]["partial"].reshape(())))
    return np.float32(total)


# revision 27
# speedup vs baseline: 1.0704x; 1.0027x over previous
"""Trainium2 Bass kernel for the masked note-accuracy loss (best-measured config)."""

import numpy as np

import concourse.bacc as bacc
import concourse.tile as tile
from concourse import mybir
from concourse.alu_op_type import AluOpType
from concourse.bass_utils import run_bass_kernel_spmd

N, T, D = 128, 2048, 88
N_CORES = 8
NS = N // N_CORES
P = 128
K = T // P

_cached_nc = None

USE_BF16_V = True
USE_NEGATE = True


def _build():
    f32 = mybir.dt.float32
    vdt = mybir.dt.bfloat16 if USE_BF16_V else f32
    nc = bacc.Bacc("TRN2", target_bir_lowering=False, debug=False,
                   num_devices=N_CORES)
    xd = nc.dram_tensor("output", [NS, T, D], f32, kind="ExternalInput")
    yd = nc.dram_tensor("target", [NS, T, D], f32, kind="ExternalInput")
    md = nc.dram_tensor("mask", [NS, T], mybir.dt.int32, kind="ExternalInput")
    od = nc.dram_tensor("partial", [1, 1], f32, kind="ExternalOutput")

    AX = mybir.AxisListType.X

    with tile.TileContext(nc) as tc:
        with (
            tc.tile_pool(name="data", bufs=3) as data_pool,
            tc.tile_pool(name="work", bufs=2) as work_pool,
            tc.tile_pool(name="mini", bufs=2) as mini_pool,
            tc.tile_pool(name="singles", bufs=1) as singles,
            tc.tile_pool(name="psl", bufs=2, space="PSUM") as psum_loop,
            tc.tile_pool(name="psk", bufs=1, space="PSUM") as psum_keep,
        ):
            stA = singles.tile([P, NS, 16], f32)
            stPQ = singles.tile([P, NS], f32)
            stQ2 = singles.tile([P, NS], f32)
            maskf = singles.tile([P, NS, K], f32)
            maski = singles.tile([P, NS, K], mybir.dt.int32)
            ones_col = singles.tile([P, 1], f32)
            ones_row = singles.tile([1, P], f32)
            inv_ti = singles.tile([1, NS], f32)
            row_ti = singles.tile([1, NS], f32)
            nc.vector.memset(ones_col[:], 1.0)
            nc.vector.memset(ones_row[:], 1.0)

            ps_m = psum_keep.tile([1, NS * K], f32)

            acc0 = mini_pool.tile([1, 1], f32, tag="acc")
            acc_prev = [acc0]
            nc.vector.memset(acc_prev[0][:], 0.0)

            def load(n):
                xt = data_pool.tile([P, K, D], f32, tag="xt")
                yt = data_pool.tile([P, K, D], f32, tag="yt")
                nc.sync.dma_start(xt[:], xd.ap()[n].rearrange("(p k) d -> p k d", p=P))
                nc.scalar.dma_start(yt[:], yd.ap()[n].rearrange("(p k) d -> p k d", p=P))
                return xt, yt

            def compute(n, xt, yt):
                v = work_pool.tile([P, K, D], vdt, tag="v")
                nc.vector.scalar_tensor_tensor(
                    out=v[:], in0=xt[:], scalar=0.0, in1=yt[:],
                    op0=AluOpType.is_gt, op1=AluOpType.subtract,
                    accum_out=stPQ[:, n : n + 1],
                )
                nc.vector.tensor_reduce(
                    out=stA[:, n, :], in_=v[:], axis=AX, op=AluOpType.add,
                    apply_absolute_value=True, negate=USE_NEGATE,
                )
                scratch = work_pool.tile([P, K, D], vdt, tag="scratch")
                nc.scalar.activation(
                    out=scratch[:], in_=yt[:],
                    func=mybir.ActivationFunctionType.Copy, scale=2.0,
                    accum_out=stQ2[:, n : n + 1],
                )
                # ---- mini epilogue ----
                ps_st = psum_loop.tile([1, 18], f32, tag="ps_st")
                nc.tensor.matmul(ps_st[0:1, 0:16], ones_col[:], stA[:, n, :])
                nc.tensor.matmul(ps_st[0:1, 16:17], ones_col[:],
                                 stPQ[:, n : n + 1])
                nc.tensor.matmul(ps_st[0:1, 17:18], ones_col[:],
                                 stQ2[:, n : n + 1])
                row_s2 = mini_pool.tile([1, 1], f32, tag="row_s2")
                nc.vector.tensor_reduce(
                    out=row_s2[:], in_=ps_st[:], axis=AX, op=AluOpType.add)
                ps_b = psum_loop.tile([P, 1], f32, tag="ps_b")
                nc.tensor.matmul(ps_b[:], ones_row[:], row_s2[:])
                sb_b = ps_b
                den = mini_pool.tile([P, K], f32, tag="den")
                nc.vector.tensor_scalar(
                    out=den[:], in0=stA[:, n, :],
                    scalar1=-2.0, scalar2=sb_b[:], op0=AluOpType.mult,
                    op1=AluOpType.add)
                rec = mini_pool.tile([P, K], f32, tag="rec")
                nc.vector.reciprocal(rec[:], den[:])
                rat = mini_pool.tile([P, K], f32, tag="rat")
                nc.vector.scalar_tensor_tensor(
                    out=rat[:], in0=rec[:], scalar=sb_b[:],
                    in1=maskf[:, n, :],
                    op0=AluOpType.mult, op1=AluOpType.mult)
                ps_rat = psum_loop.tile([1, K], f32, tag="ps_rat")
                nc.tensor.matmul(ps_rat[:], ones_col[:], rat[:])
                row_c = mini_pool.tile([1, 1], f32, tag="row_c")
                nc.vector.tensor_reduce(
                    out=row_c[:], in_=ps_rat[:], axis=AX, op=AluOpType.add)
                acc_new = mini_pool.tile([1, 1], f32, tag="acc")
                nc.vector.scalar_tensor_tensor(
                    out=acc_new[:], in0=row_c[:],
                    scalar=inv_ti[0:1, n : n + 1], in1=acc_prev[0][:],
                    op0=AluOpType.mult, op1=AluOpType.add)
                acc_prev[0] = acc_new

            xt0, yt0 = load(0)
            nc.gpsimd.dma_start(maski[:], md.ap().rearrange("n (p k) -> p n k", p=P))
            nc.vector.tensor_copy(maskf[:], maski[:])
            nc.tensor.matmul(ps_m[:], ones_col[:],
                             maskf[:].rearrange("p a b -> p (a b)"))
            nc.vector.tensor_reduce(
                out=row_ti[:], in_=ps_m[:].rearrange("o (a b) -> o a b", a=NS),
                axis=AX, op=AluOpType.add)
            nc.vector.reciprocal(inv_ti[:], row_ti[:])

            compute(0, xt0, yt0)
            for n in range(1, NS):
                xt, yt = load(n)
                compute(n, xt, yt)

            nc.sync.dma_start(od.ap(), acc_prev[0][:])

    nc.compile()
    return nc


def kernel(output, target, mask):
    global _cached_nc
    if _cached_nc is None:
        _cached_nc = _build()
    nc = _cached_nc
    output = np.asarray(output, dtype=np.float32)
    target = np.asarray(target, dtype=np.float32)
    mask = np.asarray(mask, dtype=np.int32)
    in_maps = []
    for c in range(N_CORES):
        sl = slice(c * NS, (c + 1) * NS)
        in_maps.append({
            "output": np.ascontiguousarray(output[sl]),
            "target": np.ascontiguousarray(target[sl]),
            "mask": np.ascontiguousarray(mask[sl]),
        })
    res = run_bass_kernel_spmd(nc, in_maps, list(range(N_CORES)))
    total = np.float32(0.0)
    for c in range(N_CORES):
        total = np.float32(total + np.float32(res.results[c]["partial"].reshape(())))
    return np.float32(total)
